# revision 5
# baseline (speedup 1.0000x reference)
"""Contrastive loss kernel v11 for Trainium2 (8 NeuronCores, Bass/Tile).

v9 -> v11: uniform half-tile pipeline. All PSUM accumulation tiles are
[128, 2, 512] (2 banks) in a 4-deep rotation, so a unit's slot is freed
by the EXP of the unit FOUR back instead of two: the post-stream slot
handoff never blocks (EXP 1.23us vs 8-MM unit 1.73us) and the EXP(ib0)
bubble at stream end disappears.  16 units of (8 DR matmuls, half-EXP
with accum row sums); ib0-5 exps DVE-added into the bf16 colsum acc,
ib6/ib7 exps feed the one-hot colsum matmuls directly.  The pc bank's
slot frees after exp(ib6A), three units before the end, so the acc-side
colsum matmuls overlap ib7's work.
"""

import os
import sys

import numpy as np

for _p in ("/root/.axon_site", "/root/.axon_site/_ro/trn_rl_repo",
           "/root/.axon_site/_ro/pypackages", "/opt/trn_rl_repo"):
    if os.path.isdir(_p) and _p not in sys.path:
        sys.path.append(_p)

import ml_dtypes

N, D = 4096, 1024
RG, CG = 4, 2             # core grid: 4 i-shards x 2 j-shards
CH_I = N // RG            # 1024 e1 rows per core
CH_J = N // CG            # 2048 e2 rows per core
KT = D // 128             # 8 contraction subtiles
QT = KT // 2              # 4 k-pair slabs
IBT = CH_I // 128         # 8 i-blocks per core
JCW = 512                 # j chunk width (one PSUM bank)
JCT = CH_J // JCW         # 4 j chunks
SC = 16.0                 # fp8 pre-scale; logits = psum * 10/SC^2
ACT_SCALE = 10.0 / (SC * SC)
NU = 2 * IBT              # 16 half-units
NROWS = NU + 1            # 15 units + 2 quarter-units

_CACHE = {}


def _legalize_waits(nc, cap=1):
    """Split >cap semaphore waits per instruction onto preceding NOPs."""
    import concourse.mybir as mybir
    nid = 0
    for f in nc.m.functions:
        for b in f.blocks:
            insts = b.instructions
            i = 0
            while i < len(insts):
                inst = insts[i]
                si = inst.sync_info
                if si is not None and si.on_wait and len(si.on_wait) > cap:
                    waits = list(si.on_wait)
                    inst.sync_info = mybir.SyncInfo(
                        on_wait=waits[-cap:], on_update=list(si.on_update))
                    excess = waits[:-cap]
                    pos = i
                    for j in range(0, len(excess), cap):
                        nop = mybir.InstNoOp(
                            name=f"I-waitnop-{nid}", ins=[], outs=[])
                        nid += 1
                        nop.engine = inst.engine
                        nop.sync_info = mybir.SyncInfo(
                            on_wait=excess[j:j + cap], on_update=[])
                        insts.insert(pos, nop)
                        pos += 1
                        i += 1
                i += 1
    return nc


def build_nc(legalize=True):
    import concourse.bass as bass
    import concourse.mybir as mybir
    import concourse.tile as tile
    from contextlib import ExitStack

    fp32 = mybir.dt.float32
    bf16 = mybir.dt.bfloat16
    fp8 = mybir.dt.float8e4
    AF = mybir.ActivationFunctionType
    DR = mybir.MatmulPerfMode.DoubleRow
    ADD = mybir.AluOpType.add

    nc = bass.Bass(trn_type="TRN2")
    e1t_d = nc.dram_tensor("e1t", [QT, 128, 2 * CH_I], fp8,
                           kind="ExternalInput")
    e2t_d = nc.dram_tensor("e2t", [QT, 128, 2 * CH_J], fp8,
                           kind="ExternalInput")
    rows_d = nc.dram_tensor("rows", [128, NROWS], fp32, kind="ExternalOutput")
    colp_d = nc.dram_tensor("colp", [JCT, JCW], fp32, kind="ExternalOutput")

    with ExitStack() as ctx:
        tc = ctx.enter_context(tile.TileContext(nc))
        res = ctx.enter_context(tc.tile_pool(name="res", bufs=1))

        e2t_sb = res.tile([128, QT, 2, CH_J], fp8)   # 16 KiB/part
        e1t_sb = res.tile([128, QT, 2, CH_I], fp8)   # 8 KiB/part
        acc = res.tile([128, JCT, JCW], bf16)        # 4 KiB/part colsum acc
        rows_sb = res.tile([128, NROWS], fp32)
        colp_sb = res.tile([JCT, JCW], fp32)
        jnk = res.tile([128, 2, JCW], fp8)           # warmup operand
        # maskc[:, jc, :] = [128, JCT] with column jc all ones: routes the
        # ones-matmul for j-chunk jc onto PSUM partition jc.
        maskc = res.tile([128, JCT, JCT], bf16)
        nc.vector.memset(jnk, 0.0)
        nc.vector.memset(maskc, 0.0)
        for jc in range(JCT):
            nc.vector.memset(maskc[:, jc, jc:jc + 1], 1.0)

        # ---- input DMAs in first-need order across the three rings ----
        nc.sync.dma_start(out=e2t_sb[:, 0], in_=e2t_d[0])
        nc.scalar.dma_start(out=e1t_sb[:, 0], in_=e1t_d[0])
        nc.sync.dma_start(out=e1t_sb[:, 1], in_=e1t_d[1])
        nc.gpsimd.dma_start(out=e2t_sb[:, 1], in_=e2t_d[1])
        nc.sync.dma_start(out=e2t_sb[:, 2], in_=e2t_d[2])
        nc.gpsimd.dma_start(out=e1t_sb[:, 2], in_=e1t_d[2])
        nc.sync.dma_start(out=e1t_sb[:, 3], in_=e1t_d[3])
        nc.scalar.dma_start(out=e2t_sb[:, 3], in_=e2t_d[3])

        exp_pool = ctx.enter_context(tc.tile_pool(name="exp", bufs=1))
        pmm = ctx.enter_context(tc.tile_pool(name="pmm", bufs=4,
                                             space="PSUM"))

        # ---- HAM warmup: junk DR matmuls, no DMA dependency ----
        for w in range(7):
            pj = pmm.tile([128, 2, JCW], fp32, tag="pl", name=f"pj{w}")
            nc.tensor.matmul(pj[:, 0, :], lhsT=jnk[:, :, 0:128],
                             rhs=jnk[:, :, :], start=True, stop=True,
                             perf_mode=DR)
        for w in range(6):
            pj = pmm.tile([128, 2, JCW], fp32, tag="pl", name=f"pjs{w}")
            nc.tensor.matmul(pj[:, 0, 0:128], lhsT=jnk[:, :, 0:128],
                             rhs=jnk[:, :, 0:128], start=True, stop=True,
                             perf_mode=DR)

        # ---- 16 half-units: 8 DR matmuls + half-EXP (+ DVE acc add) ----
        exB = []
        for u in range(NU - 1):
            ib, h = u // 2, u % 2
            isl = slice(ib * 128, (ib + 1) * 128)
            ph = pmm.tile([128, 2, JCW], fp32, tag="pl", name=f"ph{u}")
            for q in range(QT):
                lhsT = e1t_sb[:, q, :, isl]
                for j2 in range(2):
                    jc = 2 * h + j2
                    nc.tensor.matmul(
                        ph[:, j2, :], lhsT=lhsT,
                        rhs=e2t_sb[:, q, :, jc * JCW:(jc + 1) * JCW],
                        start=(q == 0), stop=(q == QT - 1), perf_mode=DR)
            if ib < IBT - 1:
                ex = exp_pool.tile([128, 2, JCW], bf16, tag="exA", bufs=3,
                                   name=f"exA{u}")
            else:
                ex = exp_pool.tile([128, 2, JCW], bf16, tag="exB", bufs=4,
                                   name=f"exB{u}")
                exB.append(ex)
            nc.scalar.activation(out=ex, in_=ph, func=AF.Exp,
                                 scale=ACT_SCALE,
                                 accum_out=rows_sb[:, u:u + 1])
            if ib < IBT - 1:
                asl = acc[:, 2 * h:2 * h + 2, :]
                if ib == 0:
                    nc.vector.tensor_copy(out=asl, in_=ex)
                else:
                    nc.vector.tensor_tensor(out=asl, in0=asl, in1=ex, op=ADD)

        # ---- ib7 jc2/jc3 as 1-bank quarter-units: the final EXP is only
        # [128, 512] so it clears before the PE queue reaches the last
        # colsum matmuls ----
        isl7 = slice((IBT - 1) * 128, IBT * 128)
        exQ = []
        for z in range(2):
            jc = 2 + z
            ph = pmm.tile([128, 2, JCW], fp32, tag="pl", name=f"phq{z}")
            for q in range(QT):
                nc.tensor.matmul(
                    ph[:, 0, :], lhsT=e1t_sb[:, q, :, isl7],
                    rhs=e2t_sb[:, q, :, jc * JCW:(jc + 1) * JCW],
                    start=(q == 0), stop=(q == QT - 1), perf_mode=DR)
            ex = exp_pool.tile([128, JCW], bf16, tag="exQ", bufs=2,
                               name=f"exQ{z}")
            nc.scalar.activation(out=ex, in_=ph[:, 0, :], func=AF.Exp,
                                 scale=ACT_SCALE,
                                 accum_out=rows_sb[:, NU - 1 + z:NU + z])
            exQ.append(ex)

        # ---- colsum: one accumulation group into pc ----
        pcb = pmm.tile([128, 2, JCW], fp32, tag="pl", name="pcb")
        pc = pcb[0:JCT, 0, :]
        cs = ([(acc[:, jc, :], jc) for jc in range(JCT)]
              + [(exB[0][:, j2, :], j2) for j2 in range(2)]
              + [(exQ[0], 2), (exQ[1], 3)])
        for i, (rhs, jc) in enumerate(cs):
            nc.tensor.matmul(pc, lhsT=maskc[:, jc, :], rhs=rhs,
                             start=(i == 0), stop=(i == len(cs) - 1),
                             skip_group_check=True)

        nc.scalar.dma_start(out=rows_d[:, :], in_=rows_sb)
        nc.vector.tensor_copy(out=colp_sb, in_=pc)
        nc.sync.dma_start(out=colp_d[:, :], in_=colp_sb)
    return _legalize_waits(nc) if legalize else nc


def _get_nc():
    if "nc" not in _CACHE:
        _CACHE["nc"] = build_nc()
    return _CACHE["nc"]


def _run(in_maps, trace=False, **kw):
    from concourse.bass_utils import run_bass_kernel_spmd
    return run_bass_kernel_spmd(_get_nc(), in_maps,
                                core_ids=list(range(RG * CG)),
                                trace=trace, **kw)


def _prep(embeddings1, embeddings2):
    e1 = np.asarray(embeddings1, dtype=np.float64)
    e2 = np.asarray(embeddings2, dtype=np.float64)
    e1n = e1 / np.maximum(np.linalg.norm(e1, axis=1, keepdims=True), 1e-12)
    e2n = e2 / np.maximum(np.linalg.norm(e2, axis=1, keepdims=True), 1e-12)
    ldiag = 10.0 * np.einsum("nd,nd->n", e1n, e2n)
    fp8 = ml_dtypes.float8_e4m3
    q1 = (e1n * SC).astype(np.float32).astype(fp8)
    q2 = (e2n * SC).astype(np.float32).astype(fp8)
    # k-pair slabs [QT, 128, 2*CH] (k = q*256 + half*128 + part)
    def kpair(q, ch):
        return np.ascontiguousarray(
            q.T.reshape(QT, 2, 128, ch).transpose(0, 2, 1, 3)
            .reshape(QT, 128, 2 * ch))
    e1ts = [kpair(q1[r * CH_I:(r + 1) * CH_I], CH_I) for r in range(RG)]
    e2ts = [kpair(q2[c * CH_J:(c + 1) * CH_J], CH_J) for c in range(CG)]
    return e1ts, e2ts, ldiag


def kernel(embeddings1, embeddings2, _trace=False, _full_result=False):
    e1ts, e2ts, ldiag = _prep(embeddings1, embeddings2)
    in_maps = [{"e1t": e1ts[k // CG], "e2t": e2ts[k % CG]}
               for k in range(RG * CG)]
    bres = _run(in_maps, trace=_trace)
    outs = bres.results

    rows = np.zeros(N, dtype=np.float64)
    colsum = np.zeros(N, dtype=np.float64)
    for k, o in enumerate(outs):
        r, c = k // CG, k % CG
        rr = np.asarray(o["rows"], dtype=np.float64)   # [128, NROWS]
        for ib in range(IBT - 1):
            rows[r * CH_I + ib * 128:r * CH_I + (ib + 1) * 128] += (
                rr[:, 2 * ib] + rr[:, 2 * ib + 1])
        rows[r * CH_I + (IBT - 1) * 128:(r + 1) * CH_I] += (
            rr[:, 2 * IBT - 2:].sum(axis=1))
        cp = np.asarray(o["colp"], dtype=np.float64).reshape(-1)
        colsum[c * CH_J:(c + 1) * CH_J] += cp

    ed = np.exp(ldiag)
    row_denom = rows - ed
    col_denom = colsum - ed
    sim12 = float(np.sum(ldiag - np.log(row_denom)))
    sim21 = float(np.sum(ldiag - np.log(col_denom)))
    result = (np.float32(-sim12), np.float32(-sim21))
    if _full_result:
        return result, bres
    return result


# revision 6
# speedup vs baseline: 1.0515x; 1.0515x over previous
"""Contrastive loss kernel v11 for Trainium2 (8 NeuronCores, Bass/Tile).

v9 -> v11: uniform half-tile pipeline. All PSUM accumulation tiles are
[128, 2, 512] (2 banks) in a 4-deep rotation, so a unit's slot is freed
by the EXP of the unit FOUR back instead of two: the post-stream slot
handoff never blocks (EXP 1.23us vs 8-MM unit 1.73us) and the EXP(ib0)
bubble at stream end disappears.  16 units of (8 DR matmuls, half-EXP
with accum row sums); ib0-5 exps DVE-added into the bf16 colsum acc,
ib6/ib7 exps feed the one-hot colsum matmuls directly.  The pc bank's
slot frees after exp(ib6A), three units before the end, so the acc-side
colsum matmuls overlap ib7's work.
"""

import os
import sys

import numpy as np

for _p in ("/root/.axon_site", "/root/.axon_site/_ro/trn_rl_repo",
           "/root/.axon_site/_ro/pypackages", "/opt/trn_rl_repo"):
    if os.path.isdir(_p) and _p not in sys.path:
        sys.path.append(_p)

import ml_dtypes

N, D = 4096, 1024
RG, CG = 4, 2             # core grid: 4 i-shards x 2 j-shards
CH_I = N // RG            # 1024 e1 rows per core
CH_J = N // CG            # 2048 e2 rows per core
KT = D // 128             # 8 contraction subtiles
QT = KT // 2              # 4 k-pair slabs
IBT = CH_I // 128         # 8 i-blocks per core
JCW = 512                 # j chunk width (one PSUM bank)
JCT = CH_J // JCW         # 4 j chunks
SC = 16.0                 # fp8 pre-scale; logits = psum * 10/SC^2
ACT_SCALE = 10.0 / (SC * SC)
NU = 2 * IBT              # 16 half-units
NROWS = NU + 1            # 15 units + 2 quarter-units

_CACHE = {}


def _legalize_waits(nc, cap=1):
    """Split >cap semaphore waits per instruction onto preceding NOPs."""
    import concourse.mybir as mybir
    nid = 0
    for f in nc.m.functions:
        for b in f.blocks:
            insts = b.instructions
            i = 0
            while i < len(insts):
                inst = insts[i]
                si = inst.sync_info
                if si is not None and si.on_wait and len(si.on_wait) > cap:
                    waits = list(si.on_wait)
                    inst.sync_info = mybir.SyncInfo(
                        on_wait=waits[-cap:], on_update=list(si.on_update))
                    excess = waits[:-cap]
                    pos = i
                    for j in range(0, len(excess), cap):
                        nop = mybir.InstNoOp(
                            name=f"I-waitnop-{nid}", ins=[], outs=[])
                        nid += 1
                        nop.engine = inst.engine
                        nop.sync_info = mybir.SyncInfo(
                            on_wait=excess[j:j + cap], on_update=[])
                        insts.insert(pos, nop)
                        pos += 1
                        i += 1
                i += 1
    return nc


def build_nc(legalize=True):
    import concourse.bass as bass
    import concourse.mybir as mybir
    import concourse.tile as tile
    from contextlib import ExitStack

    fp32 = mybir.dt.float32
    bf16 = mybir.dt.bfloat16
    fp8 = mybir.dt.float8e4
    AF = mybir.ActivationFunctionType
    DR = mybir.MatmulPerfMode.DoubleRow
    ADD = mybir.AluOpType.add

    nc = bass.Bass(trn_type="TRN2")
    e1t_d = nc.dram_tensor("e1t", [QT, 128, 2 * CH_I], fp8,
                           kind="ExternalInput")
    e2t_d = nc.dram_tensor("e2t", [QT, 128, 2 * CH_J], fp8,
                           kind="ExternalInput")
    rows_d = nc.dram_tensor("rows", [128, NROWS], fp32, kind="ExternalOutput")
    colp_d = nc.dram_tensor("colp", [JCT, JCW], fp32, kind="ExternalOutput")

    with ExitStack() as ctx:
        tc = ctx.enter_context(tile.TileContext(nc))
        res = ctx.enter_context(tc.tile_pool(name="res", bufs=1))

        e2t_sb = res.tile([128, QT, 2, CH_J], fp8)   # 16 KiB/part
        e1t_sb = res.tile([128, QT, 2, CH_I], fp8)   # 8 KiB/part
        acc = res.tile([128, JCT, JCW], bf16)        # 4 KiB/part colsum acc
        rows_sb = res.tile([128, NROWS], fp32)
        colp_sb = res.tile([JCT, JCW], fp32)
        jnk = res.tile([128, 2, JCW], fp8)           # warmup operand
        # maskc[:, jc, :] = [128, JCT] with column jc all ones: routes the
        # ones-matmul for j-chunk jc onto PSUM partition jc.
        maskc = res.tile([128, JCT, JCT], bf16)
        nc.vector.memset(jnk, 0.0)
        nc.vector.memset(maskc, 0.0)
        for jc in range(JCT):
            nc.vector.memset(maskc[:, jc, jc:jc + 1], 1.0)

        # ---- input DMAs in first-need order across the three rings ----
        nc.sync.dma_start(out=e2t_sb[:, 0], in_=e2t_d[0])
        nc.scalar.dma_start(out=e1t_sb[:, 0], in_=e1t_d[0])
        nc.sync.dma_start(out=e1t_sb[:, 1], in_=e1t_d[1])
        nc.gpsimd.dma_start(out=e2t_sb[:, 1], in_=e2t_d[1])
        nc.sync.dma_start(out=e2t_sb[:, 2], in_=e2t_d[2])
        nc.gpsimd.dma_start(out=e1t_sb[:, 2], in_=e1t_d[2])
        nc.sync.dma_start(out=e1t_sb[:, 3], in_=e1t_d[3])
        nc.scalar.dma_start(out=e2t_sb[:, 3], in_=e2t_d[3])

        exp_pool = ctx.enter_context(tc.tile_pool(name="exp", bufs=1))
        pmm = ctx.enter_context(tc.tile_pool(name="pmm", bufs=4,
                                             space="PSUM"))

        # ---- HAM warmup: junk DR matmuls, no DMA dependency ----
        for w in range(7):
            pj = pmm.tile([128, 2, JCW], fp32, tag="pl", name=f"pj{w}")
            nc.tensor.matmul(pj[:, 0, :], lhsT=jnk[:, :, 0:128],
                             rhs=jnk[:, :, :], start=True, stop=True,
                             perf_mode=DR)
        for w in range(6):
            pj = pmm.tile([128, 2, JCW], fp32, tag="pl", name=f"pjs{w}")
            nc.tensor.matmul(pj[:, 0, 0:128], lhsT=jnk[:, :, 0:128],
                             rhs=jnk[:, :, 0:128], start=True, stop=True,
                             perf_mode=DR)

        # ---- 16 half-units: 8 DR matmuls + half-EXP (+ DVE acc add) ----
        exB = []
        for u in range(NU - 1):
            ib, h = u // 2, u % 2
            isl = slice(ib * 128, (ib + 1) * 128)
            ph = pmm.tile([128, 2, JCW], fp32, tag="pl", name=f"ph{u}")
            for q in range(QT):
                lhsT = e1t_sb[:, q, :, isl]
                for j2 in range(2):
                    jc = 2 * h + j2
                    nc.tensor.matmul(
                        ph[:, j2, :], lhsT=lhsT,
                        rhs=e2t_sb[:, q, :, jc * JCW:(jc + 1) * JCW],
                        start=(q == 0), stop=(q == QT - 1), perf_mode=DR)
            if ib < IBT - 1:
                ex = exp_pool.tile([128, 2, JCW], bf16, tag="exA", bufs=3,
                                   name=f"exA{u}")
            else:
                ex = exp_pool.tile([128, 2, JCW], bf16, tag="exB", bufs=4,
                                   name=f"exB{u}")
                exB.append(ex)
            nc.scalar.activation(out=ex, in_=ph, func=AF.Exp,
                                 scale=ACT_SCALE,
                                 accum_out=rows_sb[:, u:u + 1])
            if ib < IBT - 1:
                asl = acc[:, 2 * h:2 * h + 2, :]
                if ib == 0:
                    nc.vector.tensor_copy(out=asl, in_=ex)
                else:
                    nc.vector.tensor_tensor(out=asl, in0=asl, in1=ex, op=ADD)

        # ---- ib7 jc2/jc3 as 1-bank quarter-units: the final EXP is only
        # [128, 512] so it clears before the PE queue reaches the last
        # colsum matmuls ----
        isl7 = slice((IBT - 1) * 128, IBT * 128)
        exQ = []
        for z in range(2):
            jc = 2 + z
            ph = pmm.tile([128, 2, JCW], fp32, tag="pl", name=f"phq{z}")
            for q in range(QT):
                nc.tensor.matmul(
                    ph[:, 0, :], lhsT=e1t_sb[:, q, :, isl7],
                    rhs=e2t_sb[:, q, :, jc * JCW:(jc + 1) * JCW],
                    start=(q == 0), stop=(q == QT - 1), perf_mode=DR)
            ex = exp_pool.tile([128, JCW], bf16, tag="exQ", bufs=2,
                               name=f"exQ{z}")
            if z == 0:
                nc.scalar.activation(out=ex, in_=ph[:, 0, :], func=AF.Exp,
                                     scale=ACT_SCALE,
                                     accum_out=rows_sb[:, NU - 1:NU])
            else:
                # no accum read: the EXP sem fires 182ns earlier, unblocking
                # the final colsum matmuls; row sums via idle-DVE reduce
                nc.scalar.activation(out=ex, in_=ph[:, 0, :], func=AF.Exp,
                                     scale=ACT_SCALE)
                nc.vector.tensor_reduce(out=rows_sb[:, NU:NU + 1], in_=ex,
                                        axis=mybir.AxisListType.X, op=ADD)
            exQ.append(ex)

        # ---- colsum: one accumulation group into pc ----
        pcb = pmm.tile([128, 2, JCW], fp32, tag="pl", name="pcb")
        pc = pcb[0:JCT, 0, :]
        cs = ([(acc[:, jc, :], jc) for jc in range(JCT)]
              + [(exB[0][:, j2, :], j2) for j2 in range(2)]
              + [(exQ[0], 2), (exQ[1], 3)])
        for i, (rhs, jc) in enumerate(cs):
            nc.tensor.matmul(pc, lhsT=maskc[:, jc, :], rhs=rhs,
                             start=(i == 0), stop=(i == len(cs) - 1),
                             skip_group_check=True)

        nc.scalar.dma_start(out=rows_d[:, :], in_=rows_sb)
        nc.vector.tensor_copy(out=colp_sb, in_=pc)
        nc.sync.dma_start(out=colp_d[:, :], in_=colp_sb)
    return _legalize_waits(nc) if legalize else nc


def _get_nc():
    if "nc" not in _CACHE:
        _CACHE["nc"] = build_nc()
    return _CACHE["nc"]


def _run(in_maps, trace=False, **kw):
    from concourse.bass_utils import run_bass_kernel_spmd
    return run_bass_kernel_spmd(_get_nc(), in_maps,
                                core_ids=list(range(RG * CG)),
                                trace=trace, **kw)


def _prep(embeddings1, embeddings2):
    e1 = np.asarray(embeddings1, dtype=np.float64)
    e2 = np.asarray(embeddings2, dtype=np.float64)
    e1n = e1 / np.maximum(np.linalg.norm(e1, axis=1, keepdims=True), 1e-12)
    e2n = e2 / np.maximum(np.linalg.norm(e2, axis=1, keepdims=True), 1e-12)
    ldiag = 10.0 * np.einsum("nd,nd->n", e1n, e2n)
    fp8 = ml_dtypes.float8_e4m3
    q1 = (e1n * SC).astype(np.float32).astype(fp8)
    q2 = (e2n * SC).astype(np.float32).astype(fp8)
    # k-pair slabs [QT, 128, 2*CH] (k = q*256 + half*128 + part)
    def kpair(q, ch):
        return np.ascontiguousarray(
            q.T.reshape(QT, 2, 128, ch).transpose(0, 2, 1, 3)
            .reshape(QT, 128, 2 * ch))
    e1ts = [kpair(q1[r * CH_I:(r + 1) * CH_I], CH_I) for r in range(RG)]
    e2ts = [kpair(q2[c * CH_J:(c + 1) * CH_J], CH_J) for c in range(CG)]
    return e1ts, e2ts, ldiag


def kernel(embeddings1, embeddings2, _trace=False, _full_result=False):
    e1ts, e2ts, ldiag = _prep(embeddings1, embeddings2)
    in_maps = [{"e1t": e1ts[k // CG], "e2t": e2ts[k % CG]}
               for k in range(RG * CG)]
    bres = _run(in_maps, trace=_trace)
    outs = bres.results

    rows = np.zeros(N, dtype=np.float64)
    colsum = np.zeros(N, dtype=np.float64)
    for k, o in enumerate(outs):
        r, c = k // CG, k % CG
        rr = np.asarray(o["rows"], dtype=np.float64)   # [128, NROWS]
        for ib in range(IBT - 1):
            rows[r * CH_I + ib * 128:r * CH_I + (ib + 1) * 128] += (
                rr[:, 2 * ib] + rr[:, 2 * ib + 1])
        rows[r * CH_I + (IBT - 1) * 128:(r + 1) * CH_I] += (
            rr[:, 2 * IBT - 2:].sum(axis=1))
        cp = np.asarray(o["colp"], dtype=np.float64).reshape(-1)
        colsum[c * CH_J:(c + 1) * CH_J] += cp

    ed = np.exp(ldiag)
    row_denom = rows - ed
    col_denom = colsum - ed
    sim12 = float(np.sum(ldiag - np.log(row_denom)))
    sim21 = float(np.sum(ldiag - np.log(col_denom)))
    result = (np.float32(-sim12), np.float32(-sim21))
    if _full_result:
        return result, bres
    return result
